# revision 16
# baseline (speedup 1.0000x reference)
"""GNN message passing (gather + segment-sum) on 8 Trainium2 NeuronCores.

Strategy (node-range sharding per the spec's sharding_hint):
  - Destination nodes are degree-balanced across the 8 cores (12500 nodes
    each), so each core owns a disjoint slice of the output and no
    cross-core reduction is needed.
  - The device-side gather uses the batched SWDGE row-gather
    (`nc.gpsimd.dma_gather`).  Its indices are int16, so x is packed as
    [25128, 256] (4 node rows per packed row plus zero rows) and
    source nodes are split into 4 "colors" by src % 4; color q gathers
    from column slice q*64:(q+1)*64 with elem_step=256 and index
    src//4 <= 25000.
  - Per core and color, the core's nodes are sorted by color-in-degree
    and grouped into 98 tiles of 128 (one SBUF partition per node).
    The slot schedule is K-MAJOR: round k touches the prefix of tiles
    whose max in-tile degree exceeds k, so consecutive slots are
    consecutive tiles and the accumulation is a handful of WIDE vector
    ops per gather call.
  - Round 0 gathers write DIRECTLY into the fp32 accumulator (no vector
    copy); rounds k>=1 gather into a staging tile and are added in.
  - Calls carry up to 32 slots (4096 descriptors = the full per-queue
    SWDGE ring at dynamic_dma_scratch_size=65536) to amortize the ~2us
    fixed SWDGE call overhead; call sizes are exact (no slot padding)
    and round-robin the 4 SWDGE queues so desc-gen of one call overlaps
    the SDMA drain of the previous ones.
  - Colors run sequentially and share a 2-deep accumulator pool; each
    color's finished tile ranges convert to bf16 and stream out to DRAM
    as their last round completes.  The host undoes the per-color
    degree-sort permutations, sums the color partials in fp32, and
    concatenates the 8 node-range slices.
"""

import numpy as np
from contextlib import ExitStack

import concourse.bacc as bacc
import concourse.bass as bass
import concourse.tile as tile
import concourse.mybir as mybir
from concourse.bass_utils import run_bass_kernel_spmd

N_NODES = 100000
N_EDGES = 1250000
D = 64
N_CORES = 8
NPC = N_NODES // N_CORES          # 12500 nodes per core
P = 128
TILES = (NPC + P - 1) // P        # 98 node tiles per core
NPC_PAD = TILES * P               # 12544
COLORS = 4
NZROWS = 128                      # zero rows dummies spread over (avoids
                                  # hotspotting one HBM line with pad reads)
RPACK = N_NODES // COLORS + NZROWS  # 25128 packed rows (tail = zeros)
DUMMY = N_NODES // COLORS         # first zero row
S_MAX = 64                        # max gather slots per dma_gather call
                                  # (ring demand is num_idxs/16+1 = 513 of the
                                  # 2047 usable per-engine ring slots, so ~3
                                  # calls stay in flight on the single queue)
SCRATCH = 32768                   # SWDGE ring carveout (2048 desc/engine)
SINGLE_PACKET = False             # single_packet coalesces a call into ONE
                                  # SDMA packet per engine, capped at 16KB =
                                  # 1024 idxs/call; False lifts the cap
STORE_MIN_TILES = 12              # min tile chunk for incremental writeback
DIRECT_ACC0 = True                # round-0 gathers write straight into acc

# Set by test.py for profiling; harness path leaves these untouched.
PROFILE = False
TRACE_CORES = None
LAST_EXEC_NS = None
LAST_RESULTS = None

_COMPILE_CACHE = {}


def _make_calls(K_q):
    """Call schedule for one color given its per-tile round counts K_q.

    Returns a list of calls; each call is a dict:
      kind: 'acc0' (round-0, gathers straight into the accumulator) or 'g'
      t0:   first tile (acc0 only)
      n:    slot count (descriptors = n*128)
      runs: [(k, t0, L, j0)] vector-add runs (g only)
    The slot order must match the idx-table row order (k-major).
    """
    K_q = np.asarray(K_q)
    kmax = int(K_q[0]) if len(K_q) else 0
    calls = []
    k_lo = 1 if DIRECT_ACC0 else 0
    if DIRECT_ACC0:
        # round 0: direct-to-acc calls over tiles [0, n0)
        n0 = int(np.count_nonzero(K_q > 0))
        t = 0
        while t < n0:
            n = min(S_MAX, n0 - t)
            calls.append(dict(kind="acc0", t0=t, n=n, runs=None))
            t += n
    # rounds k >= k_lo, k-major slot list
    slots = []
    for k in range(k_lo, kmax):
        n_k = int(np.count_nonzero(K_q > k))
        slots.extend((k, t) for t in range(n_k))
    i = 0
    while i < len(slots):
        n = min(S_MAX, len(slots) - i)
        chunk = slots[i:i + n]
        runs = []
        j = 0
        while j < n:
            k, t0 = chunk[j]
            j0 = j
            while j + 1 < n and chunk[j + 1] == (k, chunk[j][1] + 1):
                j += 1
            runs.append((k, t0, j - j0 + 1, j0))
            j += 1
        calls.append(dict(kind="g", t0=None, n=n, runs=runs))
        i += n
    return calls


def _preprocess(edge_index, x):
    """Host-side sharding: per-core, per-color padded gather-index tables."""
    dest = np.asarray(edge_index[0]).astype(np.int64)
    src = np.asarray(edge_index[1]).astype(np.int64)
    x = np.ascontiguousarray(np.asarray(x), dtype=np.float32)

    x_pack = np.zeros((RPACK, COLORS * D), np.float32)
    x_pack[:N_NODES // COLORS] = x.reshape(N_NODES // COLORS, COLORS * D)

    # Degree-balanced dest->core assignment: dests ranked by total in-degree
    # round-robin across cores, so every core sees the same degree profile
    # and the shared (max-over-cores) K schedule stays tight.
    total_deg = np.bincount(dest, minlength=N_NODES)
    rank = np.argsort(-total_deg, kind="stable")
    core_of_node = np.empty(N_NODES, np.int64)
    core_of_node[rank] = np.arange(N_NODES) % N_CORES
    dest_lists = [np.flatnonzero(core_of_node == c) for c in range(N_CORES)]
    local_id = np.empty(N_NODES, np.int64)
    for c in range(N_CORES):
        local_id[dest_lists[c]] = np.arange(len(dest_lists[c]))
    core_of = core_of_node[dest]
    # per (core, color): (perm, deg_pad, starts_pad, srcs_sorted)
    pc = [[None] * COLORS for _ in range(N_CORES)]
    K_all = np.zeros((N_CORES, COLORS, TILES), np.int64)
    for c in range(N_CORES):
        m = core_of == c
        d_loc = local_id[dest[m]]
        s_c = src[m]
        color = s_c % COLORS
        for q in range(COLORS):
            mq = color == q
            d_q = d_loc[mq]
            s_q = (s_c[mq] // COLORS).astype(np.int16)
            deg = np.bincount(d_q, minlength=NPC)
            order = np.argsort(d_q, kind="stable")
            s_sorted = s_q[order]
            starts = np.zeros(NPC, np.int64)
            starts[1:] = np.cumsum(deg)[:-1]
            perm = np.argsort(-deg, kind="stable")
            deg_pad = np.concatenate([deg[perm],
                                      np.zeros(NPC_PAD - NPC, np.int64)])
            starts_pad = np.concatenate([starts[perm],
                                         np.zeros(NPC_PAD - NPC, np.int64)])
            K_all[c, q] = deg_pad.reshape(TILES, P)[:, 0]
            pc[c][q] = (perm, deg_pad, starts_pad, s_sorted)

    K = K_all.max(axis=0)                      # [COLORS, TILES] shared schedule

    calls = [_make_calls(K[q]) for q in range(COLORS)]

    # Per-core idx tables: k-major rows per color (round 0 first), matching
    # the call schedule's slot order exactly (no padding slots).
    idx_maps = []
    n_slots_q = [int(K[q].sum()) for q in range(COLORS)]
    for c in range(N_CORES):
        parts = []
        for q in range(COLORS):
            perm, deg_pad, starts_pad, s_sorted = pc[c][q]
            kmax = int(K[q][0])
            if kmax == 0:
                continue
            s_safe = np.concatenate([s_sorted, np.zeros(1, np.int16)])
            kk = np.arange(kmax)[None, :]
            dg = deg_pad[:, None]
            st = starts_pad[:, None]
            pos = np.minimum(st + kk, len(s_safe) - 1)
            spread = ((np.arange(NPC_PAD)[:, None] + kk) % NZROWS) + DUMMY
            V = np.where(kk < dg, s_safe[pos], spread).astype(np.int16)
            Vt = V.reshape(TILES, P, kmax).transpose(2, 0, 1)  # [kmax,TILES,P]
            rows = np.empty((n_slots_q[q], P), np.int16)
            r = 0
            for k in range(kmax):
                n_k = int(np.count_nonzero(K[q] > k))
                rows[r:r + n_k] = Vt[k, :n_k]
                r += n_k
            parts.append(rows)
        vals = np.concatenate(parts, axis=0)   # [total_slots, P]
        # wrap: descriptor i of a call sits at partition i%16, col i//16,
        # replicated x8 across the 128 partitions.  Calls are concatenated
        # along the free dim so one contiguous DMA per color preloads them.
        cols = []
        r = 0
        for q in range(COLORS):
            for cl in calls[q]:
                n = cl["n"]
                blk = vals[r:r + n].reshape(n * P)
                wrapped = blk.reshape(n * P // 16, 16).T   # [16, n*8]
                cols.append(np.tile(wrapped, (8, 1)))      # [128, n*8]
                r += n
        idx_maps.append(np.ascontiguousarray(np.concatenate(cols, axis=1)))

    perms = [[pc[c][q][0] for q in range(COLORS)] for c in range(N_CORES)]
    K_key = tuple(tuple(int(v) for v in K[q]) for q in range(COLORS))
    return x_pack, idx_maps, perms, K_key, calls, dest_lists


def _build_program(K, calls):
    total_cols = sum(cl["n"] * 8 for q in range(COLORS) for cl in calls[q])
    nc = bacc.Bacc("TRN2", target_bir_lowering=False, debug=False,
                   num_devices=N_CORES, num_swdge_queues=1,
                   dynamic_dma_scratch_size=SCRATCH)
    x_dram = nc.dram_tensor("x", [RPACK, COLORS * D], mybir.dt.float32,
                            kind="ExternalInput")
    idx_dram = nc.dram_tensor("idx", [P, total_cols], mybir.dt.int16,
                              kind="ExternalInput")
    out_dram = nc.dram_tensor("out", [COLORS, NPC_PAD, D], mybir.dt.bfloat16,
                              kind="ExternalOutput")

    # column offset of each call's idx block
    col_off = []
    off = 0
    for q in range(COLORS):
        offs = []
        for cl in calls[q]:
            offs.append(off)
            off += cl["n"] * 8
        col_off.append(offs)
    qcol = [col_off[q][0] for q in range(COLORS)] + [total_cols]

    with tile.TileContext(nc) as tc, ExitStack() as ctx:
        idx_pool = ctx.enter_context(tc.tile_pool(name="idx", bufs=1))
        g_pool = ctx.enter_context(tc.tile_pool(name="g", bufs=3))
        acc_pool = ctx.enter_context(tc.tile_pool(name="acc", bufs=2))
        stage_pool = ctx.enter_context(tc.tile_pool(name="stage", bufs=2))

        # Preload idx tables per color so the first gather only waits on the
        # first chunk while the rest stream in behind it.
        idx_all = idx_pool.tile([P, total_cols], mybir.dt.int16,
                                tag="idx", name="idx_all")
        for q in range(COLORS):
            a, b = qcol[q], qcol[q + 1]
            if a == b:
                continue
            nc.sync.dma_start(out=idx_all[:, a:b], in_=idx_dram.ap()[:, a:b])

        # Warm up the SWDGE gather ucode while the idx tables stream in.
        warm_idx = idx_pool.tile([P, 8], mybir.dt.int16, tag="warm_idx",
                                 name="warm_idx")
        warm_g = g_pool.tile([P, 1, D], mybir.dt.float32, tag="warm_g",
                             name="warm_g", bufs=1)
        nc.gpsimd.memset(warm_idx[:], 0)
        nc.gpsimd.dma_gather(
            out_ap=warm_g[:], in_ap=x_dram.ap()[:, 0:D],
            idxs_ap=warm_idx[:], num_idxs=P, num_idxs_reg=P,
            elem_size=D, elem_step=COLORS * D, queue_num=0,
            single_packet=SINGLE_PACKET)

        # All gathers go to SWDGE queue 0: the 8 DMASW semaphore lanes are
        # assigned in FINAL schedule order (the Tile scheduler may reorder
        # gathers), and a lane's sem must only ever be updated from one
        # queue — with a single queue that holds trivially.
        for q in range(COLORS):
            acc = acc_pool.tile([P, TILES * D], mybir.dt.float32,
                                tag="acc", name=f"acc{q}")
            # Zero-degree tail tiles are complete from the start.
            for t in range(TILES):
                if K[q][t] == 0:
                    nc.vector.memset(acc[:, bass.ts(t, D)], 0.0)

            def store_chunk(a, b):
                stage = stage_pool.tile([P, (b - a) * D], mybir.dt.bfloat16,
                                        tag="stage", name=f"st{q}_{a}_{b}")
                nc.vector.tensor_copy(stage[:], acc[:, a * D:b * D])
                nc.sync.dma_start(
                    out=out_dram.ap()[q].rearrange("(t p) d -> p t d", p=P)
                        [:, a:b],
                    in_=stage[:].rearrange("p (t d) -> p t d", d=D))

            stored_from = TILES           # acc cols >= this are stored
            qcalls = calls[q]
            for ci, cl in enumerate(qcalls):
                n = cl["n"]
                nidx = n * P
                idxs_ap = idx_all[:, col_off[q][ci]:col_off[q][ci] + n * 8]
                if cl["kind"] == "acc0":
                    t0 = cl["t0"]
                    out_ap = acc[:, t0 * D:(t0 + n) * D].rearrange(
                        "p (s d) -> p s d", d=D)
                else:
                    g = g_pool.tile([P, n, D], mybir.dt.float32, tag="g",
                                    name=f"g{q}_{ci}")
                    out_ap = g[:]
                nc.gpsimd.dma_gather(
                    out_ap=out_ap,
                    in_ap=x_dram.ap()[:, q * D:(q + 1) * D],
                    idxs_ap=idxs_ap,
                    num_idxs=nidx,
                    num_idxs_reg=nidx,
                    elem_size=D,
                    elem_step=COLORS * D,
                    queue_num=0,
                    single_packet=SINGLE_PACKET,
                )
                if cl["kind"] == "g":
                    g2 = g[:].rearrange("p s d -> p (s d)")
                    for k, t0, L, j0 in cl["runs"]:
                        src_ap = g2[:, j0 * D:(j0 + L) * D]
                        dst_ap = acc[:, t0 * D:(t0 + L) * D]
                        if k == 0:
                            nc.vector.tensor_copy(dst_ap, src_ap)
                        else:
                            nc.vector.tensor_add(dst_ap, dst_ap, src_ap)
                # Stream out tile ranges as their last round completes
                # (k-major: high tiles finish first).
                if ci + 1 == len(qcalls):
                    if stored_from > 0:
                        store_chunk(0, stored_from)
                        stored_from = 0
                else:
                    nxt = qcalls[ci + 1]
                    if nxt["kind"] == "g" and nxt["runs"]:
                        k_next = nxt["runs"][0][0]
                        done_from = int(np.count_nonzero(
                            np.array(K[q]) > k_next))
                        if stored_from - done_from >= STORE_MIN_TILES:
                            store_chunk(done_from, stored_from)
                            stored_from = done_from
    nc.compile()
    return nc


def _install_profile_shim():
    """trace=True under axon needs the NTFF hook that this image's antenv
    lacks; register the ctypes-based one from trn_agent_boot."""
    import sys, types
    import concourse.bass_utils as bu
    if "antenv.axon_hooks" not in sys.modules:
        from trn_agent_boot.trn_boot import _ntff_profile_via_ctypes
        shim = types.ModuleType("antenv.axon_hooks")
        hook = _ntff_profile_via_ctypes("/opt/axon/libaxon_pjrt.so")
        shim.get_axon_ntff_profile_hook = lambda: hook
        shim.set_axon_ntff_profile_hook = lambda h: None
        sys.modules["antenv.axon_hooks"] = shim
    bu.upload_artifacts = lambda tmpdir: f"local:{tmpdir}"


def kernel(edge_index, x):
    global LAST_EXEC_NS, LAST_RESULTS
    (x_pack, idx_maps, perms, K, calls, dest_lists) = _preprocess(edge_index, x)

    cache_key = (K, S_MAX, DIRECT_ACC0, SINGLE_PACKET)
    if cache_key not in _COMPILE_CACHE:
        _COMPILE_CACHE[cache_key] = _build_program(K, calls)
    nc = _COMPILE_CACHE[cache_key]

    in_maps = [{"x": x_pack, "idx": idx_maps[c]} for c in range(N_CORES)]
    kwargs = {}
    if PROFILE:
        _install_profile_shim()
        kwargs = dict(trace=True, trace_cores=TRACE_CORES)
    res = run_bass_kernel_spmd(nc, in_maps, core_ids=list(range(N_CORES)),
                               **kwargs)
    LAST_EXEC_NS = res.exec_time_ns
    LAST_RESULTS = res

    out = np.empty((N_NODES, D), np.float32)
    for c in range(N_CORES):
        dev = res.results[c]["out"]            # [COLORS, NPC_PAD, D] bf16
        sl = np.zeros((NPC, D), np.float32)
        for q in range(COLORS):
            tmp = np.empty((NPC, D), np.float32)
            tmp[perms[c][q]] = dev[q][:NPC].astype(np.float32)
            sl += tmp
        out[dest_lists[c]] = sl
    return out


# revision 17
# speedup vs baseline: 1.0119x; 1.0119x over previous
"""GNN message passing (gather + segment-sum) on 8 Trainium2 NeuronCores.

Strategy (node-range sharding per the spec's sharding_hint):
  - Destination nodes are degree-balanced across the 8 cores (12500 nodes
    each), so each core owns a disjoint slice of the output and no
    cross-core reduction is needed.
  - The device-side gather uses the batched SWDGE row-gather
    (`nc.gpsimd.dma_gather`).  Its indices are int16, so x is packed as
    [25128, 256] (4 node rows per packed row plus zero rows) and
    source nodes are split into 4 "colors" by src % 4; color q gathers
    from column slice q*64:(q+1)*64 with elem_step=256 and index
    src//4 <= 25000.
  - Per core and color, the core's nodes are sorted by color-in-degree
    and grouped into 98 tiles of 128 (one SBUF partition per node).
    The slot schedule is K-MAJOR: round k touches the prefix of tiles
    whose max in-tile degree exceeds k, so consecutive slots are
    consecutive tiles and the accumulation is a handful of WIDE vector
    ops per gather call.
  - Round 0 gathers write DIRECTLY into the fp32 accumulator (no vector
    copy); rounds k>=1 gather into a staging tile and are added in.
  - Calls carry up to 32 slots (4096 descriptors = the full per-queue
    SWDGE ring at dynamic_dma_scratch_size=65536) to amortize the ~2us
    fixed SWDGE call overhead; call sizes are exact (no slot padding)
    and round-robin the 4 SWDGE queues so desc-gen of one call overlaps
    the SDMA drain of the previous ones.
  - Colors run sequentially and share a 2-deep accumulator pool; each
    color's finished tile ranges convert to bf16 and stream out to DRAM
    as their last round completes.  The host undoes the per-color
    degree-sort permutations, sums the color partials in fp32, and
    concatenates the 8 node-range slices.
"""

import numpy as np
from contextlib import ExitStack

import concourse.bacc as bacc
import concourse.bass as bass
import concourse.tile as tile
import concourse.mybir as mybir
from concourse.bass_utils import run_bass_kernel_spmd

N_NODES = 100000
N_EDGES = 1250000
D = 64
N_CORES = 8
NPC = N_NODES // N_CORES          # 12500 nodes per core
P = 128
TILES = (NPC + P - 1) // P        # 98 node tiles per core
NPC_PAD = TILES * P               # 12544
COLORS = 4
NZROWS = 128                      # zero rows dummies spread over (avoids
                                  # hotspotting one HBM line with pad reads)
RPACK = N_NODES // COLORS + NZROWS  # 25128 packed rows (tail = zeros)
DUMMY = N_NODES // COLORS         # first zero row
S_MAX = 8                         # max gather slots per dma_gather call:
                                  # single_packet coalesces a call into ONE
                                  # SDMA packet per engine, capped at 16KB =
                                  # 8 slots x 128 idx x 256B / 16 engines
SCRATCH = 16384                   # SWDGE ring carveout (default)
SINGLE_PACKET = True              # False (per-desc packets) measured ~6x
                                  # slower DMA; True is required for rate
STORE_MIN_TILES = 12              # min tile chunk for incremental writeback
DIRECT_ACC0 = True                # round-0 gathers write straight into acc

# Set by test.py for profiling; harness path leaves these untouched.
PROFILE = False
TRACE_CORES = None
LAST_EXEC_NS = None
LAST_RESULTS = None

_COMPILE_CACHE = {}


def _make_calls(K_q):
    """Call schedule for one color given its per-tile round counts K_q.

    Returns a list of calls; each call is a dict:
      kind: 'acc0' (round-0, gathers straight into the accumulator) or 'g'
      t0:   first tile (acc0 only)
      n:    slot count (descriptors = n*128)
      runs: [(k, t0, L, j0)] vector-add runs (g only)
    The slot order must match the idx-table row order (k-major).
    """
    K_q = np.asarray(K_q)
    kmax = int(K_q[0]) if len(K_q) else 0
    calls = []
    k_lo = 1 if DIRECT_ACC0 else 0
    if DIRECT_ACC0:
        # round 0: direct-to-acc calls over tiles [0, n0)
        n0 = int(np.count_nonzero(K_q > 0))
        t = 0
        while t < n0:
            n = min(S_MAX, n0 - t)
            calls.append(dict(kind="acc0", t0=t, n=n, runs=None))
            t += n
    # rounds k >= k_lo, k-major slot list
    slots = []
    for k in range(k_lo, kmax):
        n_k = int(np.count_nonzero(K_q > k))
        slots.extend((k, t) for t in range(n_k))
    i = 0
    while i < len(slots):
        n = min(S_MAX, len(slots) - i)
        chunk = slots[i:i + n]
        runs = []
        j = 0
        while j < n:
            k, t0 = chunk[j]
            j0 = j
            while j + 1 < n and chunk[j + 1] == (k, chunk[j][1] + 1):
                j += 1
            runs.append((k, t0, j - j0 + 1, j0))
            j += 1
        calls.append(dict(kind="g", t0=None, n=n, runs=runs))
        i += n
    return calls


def _preprocess(edge_index, x):
    """Host-side sharding: per-core, per-color padded gather-index tables."""
    dest = np.asarray(edge_index[0]).astype(np.int64)
    src = np.asarray(edge_index[1]).astype(np.int64)
    x = np.ascontiguousarray(np.asarray(x), dtype=np.float32)

    x_pack = np.zeros((RPACK, COLORS * D), np.float32)
    x_pack[:N_NODES // COLORS] = x.reshape(N_NODES // COLORS, COLORS * D)

    # Degree-balanced dest->core assignment: dests ranked by total in-degree
    # round-robin across cores, so every core sees the same degree profile
    # and the shared (max-over-cores) K schedule stays tight.
    total_deg = np.bincount(dest, minlength=N_NODES)
    rank = np.argsort(-total_deg, kind="stable")
    core_of_node = np.empty(N_NODES, np.int64)
    core_of_node[rank] = np.arange(N_NODES) % N_CORES
    dest_lists = [np.flatnonzero(core_of_node == c) for c in range(N_CORES)]
    local_id = np.empty(N_NODES, np.int64)
    for c in range(N_CORES):
        local_id[dest_lists[c]] = np.arange(len(dest_lists[c]))
    core_of = core_of_node[dest]
    # per (core, color): (perm, deg_pad, starts_pad, srcs_sorted)
    pc = [[None] * COLORS for _ in range(N_CORES)]
    K_all = np.zeros((N_CORES, COLORS, TILES), np.int64)
    for c in range(N_CORES):
        m = core_of == c
        d_loc = local_id[dest[m]]
        s_c = src[m]
        color = s_c % COLORS
        for q in range(COLORS):
            mq = color == q
            d_q = d_loc[mq]
            s_q = (s_c[mq] // COLORS).astype(np.int16)
            deg = np.bincount(d_q, minlength=NPC)
            order = np.argsort(d_q, kind="stable")
            s_sorted = s_q[order]
            starts = np.zeros(NPC, np.int64)
            starts[1:] = np.cumsum(deg)[:-1]
            perm = np.argsort(-deg, kind="stable")
            deg_pad = np.concatenate([deg[perm],
                                      np.zeros(NPC_PAD - NPC, np.int64)])
            starts_pad = np.concatenate([starts[perm],
                                         np.zeros(NPC_PAD - NPC, np.int64)])
            K_all[c, q] = deg_pad.reshape(TILES, P)[:, 0]
            pc[c][q] = (perm, deg_pad, starts_pad, s_sorted)

    K = K_all.max(axis=0)                      # [COLORS, TILES] shared schedule

    calls = [_make_calls(K[q]) for q in range(COLORS)]

    # Per-core idx tables: k-major rows per color (round 0 first), matching
    # the call schedule's slot order exactly (no padding slots).
    idx_maps = []
    n_slots_q = [int(K[q].sum()) for q in range(COLORS)]
    for c in range(N_CORES):
        parts = []
        for q in range(COLORS):
            perm, deg_pad, starts_pad, s_sorted = pc[c][q]
            kmax = int(K[q][0])
            if kmax == 0:
                continue
            s_safe = np.concatenate([s_sorted, np.zeros(1, np.int16)])
            kk = np.arange(kmax)[None, :]
            dg = deg_pad[:, None]
            st = starts_pad[:, None]
            pos = np.minimum(st + kk, len(s_safe) - 1)
            spread = ((np.arange(NPC_PAD)[:, None] + kk) % NZROWS) + DUMMY
            V = np.where(kk < dg, s_safe[pos], spread).astype(np.int16)
            Vt = V.reshape(TILES, P, kmax).transpose(2, 0, 1)  # [kmax,TILES,P]
            rows = np.empty((n_slots_q[q], P), np.int16)
            r = 0
            for k in range(kmax):
                n_k = int(np.count_nonzero(K[q] > k))
                rows[r:r + n_k] = Vt[k, :n_k]
                r += n_k
            parts.append(rows)
        vals = np.concatenate(parts, axis=0)   # [total_slots, P]
        # wrap: descriptor i of a call sits at partition i%16, col i//16,
        # replicated x8 across the 128 partitions.  Calls are concatenated
        # along the free dim so one contiguous DMA per color preloads them.
        cols = []
        r = 0
        for q in range(COLORS):
            for cl in calls[q]:
                n = cl["n"]
                blk = vals[r:r + n].reshape(n * P)
                wrapped = blk.reshape(n * P // 16, 16).T   # [16, n*8]
                cols.append(np.tile(wrapped, (8, 1)))      # [128, n*8]
                r += n
        idx_maps.append(np.ascontiguousarray(np.concatenate(cols, axis=1)))

    perms = [[pc[c][q][0] for q in range(COLORS)] for c in range(N_CORES)]
    K_key = tuple(tuple(int(v) for v in K[q]) for q in range(COLORS))
    return x_pack, idx_maps, perms, K_key, calls, dest_lists


def _build_program(K, calls):
    total_cols = sum(cl["n"] * 8 for q in range(COLORS) for cl in calls[q])
    nc = bacc.Bacc("TRN2", target_bir_lowering=False, debug=False,
                   num_devices=N_CORES, num_swdge_queues=1,
                   dynamic_dma_scratch_size=SCRATCH)
    x_dram = nc.dram_tensor("x", [RPACK, COLORS * D], mybir.dt.float32,
                            kind="ExternalInput")
    idx_dram = nc.dram_tensor("idx", [P, total_cols], mybir.dt.int16,
                              kind="ExternalInput")
    out_dram = nc.dram_tensor("out", [COLORS, NPC_PAD, D], mybir.dt.bfloat16,
                              kind="ExternalOutput")

    # column offset of each call's idx block
    col_off = []
    off = 0
    for q in range(COLORS):
        offs = []
        for cl in calls[q]:
            offs.append(off)
            off += cl["n"] * 8
        col_off.append(offs)
    qcol = [col_off[q][0] for q in range(COLORS)] + [total_cols]

    with tile.TileContext(nc) as tc, ExitStack() as ctx:
        idx_pool = ctx.enter_context(tc.tile_pool(name="idx", bufs=1))
        g_pool = ctx.enter_context(tc.tile_pool(name="g", bufs=8))
        acc_pool = ctx.enter_context(tc.tile_pool(name="acc", bufs=2))
        stage_pool = ctx.enter_context(tc.tile_pool(name="stage", bufs=2))

        # Preload idx tables per color so the first gather only waits on the
        # first chunk while the rest stream in behind it.
        idx_all = idx_pool.tile([P, total_cols], mybir.dt.int16,
                                tag="idx", name="idx_all")
        for q in range(COLORS):
            a, b = qcol[q], qcol[q + 1]
            if a == b:
                continue
            nc.sync.dma_start(out=idx_all[:, a:b], in_=idx_dram.ap()[:, a:b])

        # Warm up the SWDGE gather ucode while the idx tables stream in.
        warm_idx = idx_pool.tile([P, 8], mybir.dt.int16, tag="warm_idx",
                                 name="warm_idx")
        warm_g = g_pool.tile([P, 1, D], mybir.dt.float32, tag="warm_g",
                             name="warm_g", bufs=1)
        nc.gpsimd.memset(warm_idx[:], 0)
        nc.gpsimd.dma_gather(
            out_ap=warm_g[:], in_ap=x_dram.ap()[:, 0:D],
            idxs_ap=warm_idx[:], num_idxs=P, num_idxs_reg=P,
            elem_size=D, elem_step=COLORS * D, queue_num=0,
            single_packet=SINGLE_PACKET)

        # All gathers go to SWDGE queue 0: the 8 DMASW semaphore lanes are
        # assigned in FINAL schedule order (the Tile scheduler may reorder
        # gathers), and a lane's sem must only ever be updated from one
        # queue — with a single queue that holds trivially.
        for q in range(COLORS):
            acc = acc_pool.tile([P, TILES * D], mybir.dt.float32,
                                tag="acc", name=f"acc{q}")
            # Zero-degree tail tiles are complete from the start.
            for t in range(TILES):
                if K[q][t] == 0:
                    nc.vector.memset(acc[:, bass.ts(t, D)], 0.0)

            def store_chunk(a, b):
                stage = stage_pool.tile([P, (b - a) * D], mybir.dt.bfloat16,
                                        tag="stage", name=f"st{q}_{a}_{b}")
                nc.vector.tensor_copy(stage[:], acc[:, a * D:b * D])
                nc.sync.dma_start(
                    out=out_dram.ap()[q].rearrange("(t p) d -> p t d", p=P)
                        [:, a:b],
                    in_=stage[:].rearrange("p (t d) -> p t d", d=D))

            stored_from = TILES           # acc cols >= this are stored
            qcalls = calls[q]
            for ci, cl in enumerate(qcalls):
                n = cl["n"]
                nidx = n * P
                idxs_ap = idx_all[:, col_off[q][ci]:col_off[q][ci] + n * 8]
                if cl["kind"] == "acc0":
                    t0 = cl["t0"]
                    out_ap = acc[:, t0 * D:(t0 + n) * D].rearrange(
                        "p (s d) -> p s d", d=D)
                else:
                    g = g_pool.tile([P, n, D], mybir.dt.float32, tag="g",
                                    name=f"g{q}_{ci}")
                    out_ap = g[:]
                nc.gpsimd.dma_gather(
                    out_ap=out_ap,
                    in_ap=x_dram.ap()[:, q * D:(q + 1) * D],
                    idxs_ap=idxs_ap,
                    num_idxs=nidx,
                    num_idxs_reg=nidx,
                    elem_size=D,
                    elem_step=COLORS * D,
                    queue_num=0,
                    single_packet=SINGLE_PACKET,
                )
                if cl["kind"] == "g":
                    g2 = g[:].rearrange("p s d -> p (s d)")
                    for k, t0, L, j0 in cl["runs"]:
                        src_ap = g2[:, j0 * D:(j0 + L) * D]
                        dst_ap = acc[:, t0 * D:(t0 + L) * D]
                        if k == 0:
                            nc.vector.tensor_copy(dst_ap, src_ap)
                        else:
                            nc.vector.tensor_add(dst_ap, dst_ap, src_ap)
                # Stream out tile ranges as their last round completes
                # (k-major: high tiles finish first).
                if ci + 1 == len(qcalls):
                    if stored_from > 0:
                        store_chunk(0, stored_from)
                        stored_from = 0
                else:
                    nxt = qcalls[ci + 1]
                    if nxt["kind"] == "g" and nxt["runs"]:
                        k_next = nxt["runs"][0][0]
                        done_from = int(np.count_nonzero(
                            np.array(K[q]) > k_next))
                        if stored_from - done_from >= STORE_MIN_TILES:
                            store_chunk(done_from, stored_from)
                            stored_from = done_from
    nc.compile()
    return nc


def _install_profile_shim():
    """trace=True under axon needs the NTFF hook that this image's antenv
    lacks; register the ctypes-based one from trn_agent_boot."""
    import sys, types
    import concourse.bass_utils as bu
    if "antenv.axon_hooks" not in sys.modules:
        from trn_agent_boot.trn_boot import _ntff_profile_via_ctypes
        shim = types.ModuleType("antenv.axon_hooks")
        hook = _ntff_profile_via_ctypes("/opt/axon/libaxon_pjrt.so")
        shim.get_axon_ntff_profile_hook = lambda: hook
        shim.set_axon_ntff_profile_hook = lambda h: None
        sys.modules["antenv.axon_hooks"] = shim
    bu.upload_artifacts = lambda tmpdir: f"local:{tmpdir}"


def kernel(edge_index, x):
    global LAST_EXEC_NS, LAST_RESULTS
    (x_pack, idx_maps, perms, K, calls, dest_lists) = _preprocess(edge_index, x)

    cache_key = (K, S_MAX, DIRECT_ACC0, SINGLE_PACKET)
    if cache_key not in _COMPILE_CACHE:
        _COMPILE_CACHE[cache_key] = _build_program(K, calls)
    nc = _COMPILE_CACHE[cache_key]

    in_maps = [{"x": x_pack, "idx": idx_maps[c]} for c in range(N_CORES)]
    kwargs = {}
    if PROFILE:
        _install_profile_shim()
        kwargs = dict(trace=True, trace_cores=TRACE_CORES)
    res = run_bass_kernel_spmd(nc, in_maps, core_ids=list(range(N_CORES)),
                               **kwargs)
    LAST_EXEC_NS = res.exec_time_ns
    LAST_RESULTS = res

    out = np.empty((N_NODES, D), np.float32)
    for c in range(N_CORES):
        dev = res.results[c]["out"]            # [COLORS, NPC_PAD, D] bf16
        sl = np.zeros((NPC, D), np.float32)
        for q in range(COLORS):
            tmp = np.empty((NPC, D), np.float32)
            tmp[perms[c][q]] = dev[q][:NPC].astype(np.float32)
            sl += tmp
        out[dest_lists[c]] = sl
    return out


# revision 21
# speedup vs baseline: 3.3784x; 3.3385x over previous
"""GNN message passing (gather + segment-sum) on 8 Trainium2 NeuronCores.

Strategy (node-range sharding per the spec's sharding_hint):
  - Destination nodes are degree-balanced across the 8 cores (12500 nodes
    each), so each core owns a disjoint slice of the output and no
    cross-core reduction is needed.
  - The device-side gather uses the batched SWDGE row-gather
    (`nc.gpsimd.dma_gather`).  Its indices are int16, so x is packed as
    [25128, 256] (4 node rows per packed row plus zero rows) and
    source nodes are split into 4 "colors" by src % 4; color q gathers
    from column slice q*64:(q+1)*64 with elem_step=256 and index
    src//4 <= 25000.
  - Per core and color, the core's nodes are sorted by color-in-degree
    and grouped into 98 tiles of 128 (one SBUF partition per node).
    The slot schedule is K-MAJOR: round k touches the prefix of tiles
    whose max in-tile degree exceeds k, so consecutive slots are
    consecutive tiles and the accumulation is a handful of WIDE vector
    ops per gather call.
  - Round 0 gathers write DIRECTLY into the fp32 accumulator (no vector
    copy); rounds k>=1 gather into a staging tile and are added in.
  - Calls carry up to 32 slots (4096 descriptors = the full per-queue
    SWDGE ring at dynamic_dma_scratch_size=65536) to amortize the ~2us
    fixed SWDGE call overhead; call sizes are exact (no slot padding)
    and round-robin the 4 SWDGE queues so desc-gen of one call overlaps
    the SDMA drain of the previous ones.
  - Colors run sequentially and share a 2-deep accumulator pool; each
    color's finished tile ranges convert to bf16 and stream out to DRAM
    as their last round completes.  The host undoes the per-color
    degree-sort permutations, sums the color partials in fp32, and
    concatenates the 8 node-range slices.
"""

import numpy as np
from contextlib import ExitStack

import concourse.bacc as bacc
import concourse.bass as bass
import concourse.tile as tile
import concourse.mybir as mybir
from concourse.bass_utils import run_bass_kernel_spmd
from concourse.instruction_name_ordered_set import InstructionNameOrderedSet

N_NODES = 100000
N_EDGES = 1250000
D = 64
N_CORES = 8
NPC = N_NODES // N_CORES          # 12500 nodes per core
P = 128
TILES = (NPC + P - 1) // P        # 98 node tiles per core
NPC_PAD = TILES * P               # 12544
COLORS = 4
NZROWS = 128                      # zero rows dummies spread over (avoids
                                  # hotspotting one HBM line with pad reads)
RPACK = N_NODES // COLORS + NZROWS  # 25128 packed rows (tail = zeros)
DUMMY = N_NODES // COLORS         # first zero row
S_MAX = 8                         # max gather slots per dma_gather call:
                                  # single_packet coalesces a call into ONE
                                  # SDMA packet per engine, capped at 16KB =
                                  # 8 slots x 128 idx x 256B / 16 engines
SCRATCH = 16384                   # SWDGE ring carveout (default)
SINGLE_PACKET = True              # False (per-desc packets) measured ~6x
                                  # slower DMA; True is required for rate
STORE_MIN_TILES = 12              # min tile chunk for incremental writeback
DIRECT_ACC0 = True                # round-0 gathers write straight into acc

# Set by test.py for profiling; harness path leaves these untouched.
PROFILE = False
TRACE_CORES = None
LAST_EXEC_NS = None
LAST_RESULTS = None

_COMPILE_CACHE = {}


def _make_calls(K_q):
    """Call schedule for one color given its per-tile round counts K_q.

    Returns a list of calls; each call is a dict:
      kind: 'acc0' (round-0, gathers straight into the accumulator) or 'g'
      t0:   first tile (acc0 only)
      n:    slot count (descriptors = n*128)
      runs: [(k, t0, L, j0)] vector-add runs (g only)
    The slot order must match the idx-table row order (k-major).
    """
    K_q = np.asarray(K_q)
    kmax = int(K_q[0]) if len(K_q) else 0
    calls = []
    k_lo = 1 if DIRECT_ACC0 else 0
    if DIRECT_ACC0:
        # round 0: direct-to-acc calls over tiles [0, n0)
        n0 = int(np.count_nonzero(K_q > 0))
        t = 0
        while t < n0:
            n = min(S_MAX, n0 - t)
            calls.append(dict(kind="acc0", t0=t, n=n, runs=None))
            t += n
    # rounds k >= k_lo, k-major slot list
    slots = []
    for k in range(k_lo, kmax):
        n_k = int(np.count_nonzero(K_q > k))
        slots.extend((k, t) for t in range(n_k))
    i = 0
    while i < len(slots):
        n = min(S_MAX, len(slots) - i)
        chunk = slots[i:i + n]
        runs = []
        j = 0
        while j < n:
            k, t0 = chunk[j]
            j0 = j
            while j + 1 < n and chunk[j + 1] == (k, chunk[j][1] + 1):
                j += 1
            runs.append((k, t0, j - j0 + 1, j0))
            j += 1
        calls.append(dict(kind="g", t0=None, n=n, runs=runs))
        i += n
    return calls


def _preprocess(edge_index, x):
    """Host-side sharding: per-core, per-color padded gather-index tables."""
    dest = np.asarray(edge_index[0]).astype(np.int64)
    src = np.asarray(edge_index[1]).astype(np.int64)
    x = np.ascontiguousarray(np.asarray(x), dtype=np.float32)

    x_pack = np.zeros((RPACK, COLORS * D), np.float32)
    x_pack[:N_NODES // COLORS] = x.reshape(N_NODES // COLORS, COLORS * D)

    # Degree-balanced dest->core assignment: dests ranked by total in-degree
    # round-robin across cores, so every core sees the same degree profile
    # and the shared (max-over-cores) K schedule stays tight.
    total_deg = np.bincount(dest, minlength=N_NODES)
    rank = np.argsort(-total_deg, kind="stable")
    core_of_node = np.empty(N_NODES, np.int64)
    core_of_node[rank] = np.arange(N_NODES) % N_CORES
    dest_lists = [np.flatnonzero(core_of_node == c) for c in range(N_CORES)]
    local_id = np.empty(N_NODES, np.int64)
    for c in range(N_CORES):
        local_id[dest_lists[c]] = np.arange(len(dest_lists[c]))
    core_of = core_of_node[dest]
    # per (core, color): (perm, deg_pad, starts_pad, srcs_sorted)
    pc = [[None] * COLORS for _ in range(N_CORES)]
    K_all = np.zeros((N_CORES, COLORS, TILES), np.int64)
    for c in range(N_CORES):
        m = core_of == c
        d_loc = local_id[dest[m]]
        s_c = src[m]
        color = s_c % COLORS
        for q in range(COLORS):
            mq = color == q
            d_q = d_loc[mq]
            s_q = (s_c[mq] // COLORS).astype(np.int16)
            deg = np.bincount(d_q, minlength=NPC)
            order = np.argsort(d_q, kind="stable")
            s_sorted = s_q[order]
            starts = np.zeros(NPC, np.int64)
            starts[1:] = np.cumsum(deg)[:-1]
            perm = np.argsort(-deg, kind="stable")
            deg_pad = np.concatenate([deg[perm],
                                      np.zeros(NPC_PAD - NPC, np.int64)])
            starts_pad = np.concatenate([starts[perm],
                                         np.zeros(NPC_PAD - NPC, np.int64)])
            K_all[c, q] = deg_pad.reshape(TILES, P)[:, 0]
            pc[c][q] = (perm, deg_pad, starts_pad, s_sorted)

    K = K_all.max(axis=0)                      # [COLORS, TILES] shared schedule

    calls = [_make_calls(K[q]) for q in range(COLORS)]

    # Per-core idx tables: k-major rows per color (round 0 first), matching
    # the call schedule's slot order exactly (no padding slots).
    idx_maps = []
    n_slots_q = [int(K[q].sum()) for q in range(COLORS)]
    for c in range(N_CORES):
        parts = []
        for q in range(COLORS):
            perm, deg_pad, starts_pad, s_sorted = pc[c][q]
            kmax = int(K[q][0])
            if kmax == 0:
                continue
            s_safe = np.concatenate([s_sorted, np.zeros(1, np.int16)])
            kk = np.arange(kmax)[None, :]
            dg = deg_pad[:, None]
            st = starts_pad[:, None]
            pos = np.minimum(st + kk, len(s_safe) - 1)
            spread = ((np.arange(NPC_PAD)[:, None] + kk) % NZROWS) + DUMMY
            V = np.where(kk < dg, s_safe[pos], spread).astype(np.int16)
            Vt = V.reshape(TILES, P, kmax).transpose(2, 0, 1)  # [kmax,TILES,P]
            rows = np.empty((n_slots_q[q], P), np.int16)
            r = 0
            for k in range(kmax):
                n_k = int(np.count_nonzero(K[q] > k))
                rows[r:r + n_k] = Vt[k, :n_k]
                r += n_k
            parts.append(rows)
        vals = np.concatenate(parts, axis=0)   # [total_slots, P]
        # wrap: descriptor i of a call sits at partition i%16, col i//16,
        # replicated x8 across the 128 partitions.  Calls are concatenated
        # along the free dim so one contiguous DMA per color preloads them.
        cols = []
        r = 0
        for q in range(COLORS):
            for cl in calls[q]:
                n = cl["n"]
                blk = vals[r:r + n].reshape(n * P)
                wrapped = blk.reshape(n * P // 16, 16).T   # [16, n*8]
                cols.append(np.tile(wrapped, (8, 1)))      # [128, n*8]
                r += n
        idx_maps.append(np.ascontiguousarray(np.concatenate(cols, axis=1)))

    perms = [[pc[c][q][0] for q in range(COLORS)] for c in range(N_CORES)]
    K_key = tuple(tuple(int(v) for v in K[q]) for q in range(COLORS))
    return x_pack, idx_maps, perms, K_key, calls, dest_lists


def _build_program(K, calls):
    total_cols = sum(cl["n"] * 8 for q in range(COLORS) for cl in calls[q])
    nc = bacc.Bacc("TRN2", target_bir_lowering=False, debug=False,
                   num_devices=N_CORES, num_swdge_queues=4,
                   dynamic_dma_scratch_size=SCRATCH)
    x_dram = nc.dram_tensor("x", [RPACK, COLORS * D], mybir.dt.float32,
                            kind="ExternalInput")
    idx_dram = nc.dram_tensor("idx", [P, total_cols], mybir.dt.int16,
                              kind="ExternalInput")
    out_dram = nc.dram_tensor("out", [COLORS, NPC_PAD, D], mybir.dt.bfloat16,
                              kind="ExternalOutput")

    # column offset of each call's idx block
    col_off = []
    off = 0
    for q in range(COLORS):
        offs = []
        for cl in calls[q]:
            offs.append(off)
            off += cl["n"] * 8
        col_off.append(offs)
    qcol = [col_off[q][0] for q in range(COLORS)] + [total_cols]

    with tile.TileContext(nc) as tc, ExitStack() as ctx:
        idx_pool = ctx.enter_context(tc.tile_pool(name="idx", bufs=1))
        g_pool = ctx.enter_context(tc.tile_pool(name="g", bufs=8))
        acc_pool = ctx.enter_context(tc.tile_pool(name="acc", bufs=2))
        stage_pool = ctx.enter_context(tc.tile_pool(name="stage", bufs=2))

        # Preload idx tables per color so the first gather only waits on the
        # first chunk while the rest stream in behind it.
        idx_all = idx_pool.tile([P, total_cols], mybir.dt.int16,
                                tag="idx", name="idx_all")
        for q in range(COLORS):
            a, b = qcol[q], qcol[q + 1]
            if a == b:
                continue
            nc.sync.dma_start(out=idx_all[:, a:b], in_=idx_dram.ap()[:, a:b])

        # Warm up the SWDGE gather ucode while the idx tables stream in.
        warm_idx = idx_pool.tile([P, 8], mybir.dt.int16, tag="warm_idx",
                                 name="warm_idx")
        warm_g = g_pool.tile([P, 1, D], mybir.dt.float32, tag="warm_g",
                             name="warm_g", bufs=1)
        nc.gpsimd.memset(warm_idx[:], 0)
        prev = nc.gpsimd.dma_gather(
            out_ap=warm_g[:], in_ap=x_dram.ap()[:, 0:D],
            idxs_ap=warm_idx[:], num_idxs=P, num_idxs_reg=P,
            elem_size=D, elem_step=COLORS * D, queue_num=0,
            single_packet=SINGLE_PACKET)

        # Descriptor generation runs ASYNCHRONOUSLY on a per-queue SWDGE
        # worker (~8.6us per 1024-idx call); the Pool engine only blocks when
        # re-issuing to a still-busy queue, so rotating all 4 queues gives 4x
        # desc-gen throughput.  The 8 DMASW semaphore lanes are assigned in
        # FINAL schedule order and each lane's sem must only ever be updated
        # from one queue, so gathers are chained with no-sync deps (freezing
        # their order) and queue = (pool-DMA index) % 4, keeping lane L on
        # queue L % 4 forever.
        gi = 1                                 # warm gather was #0 (queue 0)
        for q in range(COLORS):
            acc = acc_pool.tile([P, TILES * D], mybir.dt.float32,
                                tag="acc", name=f"acc{q}")
            # Zero-degree tail tiles are complete from the start.
            for t in range(TILES):
                if K[q][t] == 0:
                    nc.vector.memset(acc[:, bass.ts(t, D)], 0.0)

            def store_chunk(a, b):
                stage = stage_pool.tile([P, (b - a) * D], mybir.dt.bfloat16,
                                        tag="stage", name=f"st{q}_{a}_{b}")
                nc.vector.tensor_copy(stage[:], acc[:, a * D:b * D])
                nc.sync.dma_start(
                    out=out_dram.ap()[q].rearrange("(t p) d -> p t d", p=P)
                        [:, a:b],
                    in_=stage[:].rearrange("p (t d) -> p t d", d=D))

            stored_from = TILES           # acc cols >= this are stored
            qcalls = calls[q]
            for ci, cl in enumerate(qcalls):
                n = cl["n"]
                nidx = n * P
                idxs_ap = idx_all[:, col_off[q][ci]:col_off[q][ci] + n * 8]
                if cl["kind"] == "acc0":
                    t0 = cl["t0"]
                    out_ap = acc[:, t0 * D:(t0 + n) * D].rearrange(
                        "p (s d) -> p s d", d=D)
                else:
                    g = g_pool.tile([P, n, D], mybir.dt.float32, tag="g",
                                    name=f"g{q}_{ci}")
                    out_ap = g[:]
                inst = nc.gpsimd.dma_gather(
                    out_ap=out_ap,
                    in_ap=x_dram.ap()[:, q * D:(q + 1) * D],
                    idxs_ap=idxs_ap,
                    num_idxs=nidx,
                    num_idxs_reg=nidx,
                    elem_size=D,
                    elem_step=COLORS * D,
                    queue_num=gi % 4,
                    single_packet=SINGLE_PACKET,
                )
                gi += 1
                deps = InstructionNameOrderedSet()
                deps.add(prev.ins.name)
                inst.ins.add_nosync_dependencies_from(deps)
                prev = inst
                if cl["kind"] == "g":
                    g2 = g[:].rearrange("p s d -> p (s d)")
                    for k, t0, L, j0 in cl["runs"]:
                        src_ap = g2[:, j0 * D:(j0 + L) * D]
                        dst_ap = acc[:, t0 * D:(t0 + L) * D]
                        if k == 0:
                            nc.vector.tensor_copy(dst_ap, src_ap)
                        else:
                            nc.vector.tensor_add(dst_ap, dst_ap, src_ap)
                # Stream out tile ranges as their last round completes
                # (k-major: high tiles finish first).
                if ci + 1 == len(qcalls):
                    if stored_from > 0:
                        store_chunk(0, stored_from)
                        stored_from = 0
                else:
                    nxt = qcalls[ci + 1]
                    if nxt["kind"] == "g" and nxt["runs"]:
                        k_next = nxt["runs"][0][0]
                        done_from = int(np.count_nonzero(
                            np.array(K[q]) > k_next))
                        if stored_from - done_from >= STORE_MIN_TILES:
                            store_chunk(done_from, stored_from)
                            stored_from = done_from
    nc.compile()
    return nc


def _install_profile_shim():
    """trace=True under axon needs the NTFF hook that this image's antenv
    lacks; register the ctypes-based one from trn_agent_boot."""
    import sys, types
    import concourse.bass_utils as bu
    if "antenv.axon_hooks" not in sys.modules:
        from trn_agent_boot.trn_boot import _ntff_profile_via_ctypes
        shim = types.ModuleType("antenv.axon_hooks")
        hook = _ntff_profile_via_ctypes("/opt/axon/libaxon_pjrt.so")
        shim.get_axon_ntff_profile_hook = lambda: hook
        shim.set_axon_ntff_profile_hook = lambda h: None
        sys.modules["antenv.axon_hooks"] = shim
    bu.upload_artifacts = lambda tmpdir: f"local:{tmpdir}"


def kernel(edge_index, x):
    global LAST_EXEC_NS, LAST_RESULTS
    (x_pack, idx_maps, perms, K, calls, dest_lists) = _preprocess(edge_index, x)

    cache_key = (K, S_MAX, DIRECT_ACC0, SINGLE_PACKET)
    if cache_key not in _COMPILE_CACHE:
        _COMPILE_CACHE[cache_key] = _build_program(K, calls)
    nc = _COMPILE_CACHE[cache_key]

    in_maps = [{"x": x_pack, "idx": idx_maps[c]} for c in range(N_CORES)]
    kwargs = {}
    if PROFILE:
        _install_profile_shim()
        kwargs = dict(trace=True, trace_cores=TRACE_CORES)
    res = run_bass_kernel_spmd(nc, in_maps, core_ids=list(range(N_CORES)),
                               **kwargs)
    LAST_EXEC_NS = res.exec_time_ns
    LAST_RESULTS = res

    out = np.empty((N_NODES, D), np.float32)
    for c in range(N_CORES):
        dev = res.results[c]["out"]            # [COLORS, NPC_PAD, D] bf16
        sl = np.zeros((NPC, D), np.float32)
        for q in range(COLORS):
            tmp = np.empty((NPC, D), np.float32)
            tmp[perms[c][q]] = dev[q][:NPC].astype(np.float32)
            sl += tmp
        out[dest_lists[c]] = sl
    return out


# revision 22
# speedup vs baseline: 3.5964x; 1.0645x over previous
"""GNN message passing (gather + segment-sum) on 8 Trainium2 NeuronCores.

Strategy (node-range sharding per the spec's sharding_hint):
  - Destination nodes are degree-balanced across the 8 cores (12500 nodes
    each), so each core owns a disjoint slice of the output and no
    cross-core reduction is needed.
  - The device-side gather uses the batched SWDGE row-gather
    (`nc.gpsimd.dma_gather`).  Its indices are int16, so x is packed as
    [25128, 256] (4 node rows per packed row plus zero rows) and
    source nodes are split into 4 "colors" by src % 4; color q gathers
    from column slice q*64:(q+1)*64 with elem_step=256 and index
    src//4 <= 25000.
  - Per core and color, the core's nodes are sorted by color-in-degree
    and grouped into 98 tiles of 128 (one SBUF partition per node).
    The slot schedule is K-MAJOR: round k touches the prefix of tiles
    whose max in-tile degree exceeds k, so consecutive slots are
    consecutive tiles and the accumulation is a handful of WIDE vector
    ops per gather call.
  - Round 0 gathers write DIRECTLY into the fp32 accumulator (no vector
    copy); rounds k>=1 gather into a staging tile and are added in.
  - Calls carry up to 32 slots (4096 descriptors = the full per-queue
    SWDGE ring at dynamic_dma_scratch_size=65536) to amortize the ~2us
    fixed SWDGE call overhead; call sizes are exact (no slot padding)
    and round-robin the 4 SWDGE queues so desc-gen of one call overlaps
    the SDMA drain of the previous ones.
  - Colors run sequentially and share a 2-deep accumulator pool; each
    color's finished tile ranges convert to bf16 and stream out to DRAM
    as their last round completes.  The host undoes the per-color
    degree-sort permutations, sums the color partials in fp32, and
    concatenates the 8 node-range slices.
"""

import numpy as np
from contextlib import ExitStack

import concourse.bacc as bacc
import concourse.bass as bass
import concourse.tile as tile
import concourse.mybir as mybir
from concourse.bass_utils import run_bass_kernel_spmd
from concourse.instruction_name_ordered_set import InstructionNameOrderedSet

N_NODES = 100000
N_EDGES = 1250000
D = 64
N_CORES = 8
NPC = N_NODES // N_CORES          # 12500 nodes per core
P = 128
TILES = (NPC + P - 1) // P        # 98 node tiles per core
NPC_PAD = TILES * P               # 12544
COLORS = 4
NZROWS = 128                      # zero rows dummies spread over (avoids
                                  # hotspotting one HBM line with pad reads)
RPACK = N_NODES // COLORS + NZROWS  # 25128 packed rows (tail = zeros)
DUMMY = N_NODES // COLORS         # first zero row
S_MAX = 8                         # max gather slots per dma_gather call:
                                  # single_packet coalesces a call into ONE
                                  # SDMA packet per engine, capped at 16KB =
                                  # 8 slots x 128 idx x 256B / 16 engines
SCRATCH = 16384                   # SWDGE ring carveout (default)
SINGLE_PACKET = True              # False (per-desc packets) measured ~6x
                                  # slower DMA; True is required for rate
STORE_MIN_TILES = 12              # min tile chunk for incremental writeback
DIRECT_ACC0 = True                # round-0 gathers write straight into acc

# Set by test.py for profiling; harness path leaves these untouched.
PROFILE = False
TRACE_CORES = None
LAST_EXEC_NS = None
LAST_RESULTS = None

_COMPILE_CACHE = {}


def _make_calls(K_q):
    """Call schedule for one color given its per-tile round counts K_q.

    Returns a list of calls; each call is a dict:
      kind: 'acc0' (round-0, gathers straight into the accumulator) or 'g'
      t0:   first tile (acc0 only)
      n:    slot count (descriptors = n*128)
      runs: [(k, t0, L, j0)] vector-add runs (g only)
    The slot order must match the idx-table row order (k-major).
    """
    K_q = np.asarray(K_q)
    kmax = int(K_q[0]) if len(K_q) else 0
    calls = []
    k_lo = 1 if DIRECT_ACC0 else 0
    if DIRECT_ACC0:
        # round 0: direct-to-acc calls over tiles [0, n0)
        n0 = int(np.count_nonzero(K_q > 0))
        t = 0
        while t < n0:
            n = min(S_MAX, n0 - t)
            calls.append(dict(kind="acc0", t0=t, n=n, runs=None))
            t += n
    # rounds k >= k_lo, k-major slot list
    slots = []
    for k in range(k_lo, kmax):
        n_k = int(np.count_nonzero(K_q > k))
        slots.extend((k, t) for t in range(n_k))
    i = 0
    while i < len(slots):
        n = min(S_MAX, len(slots) - i)
        chunk = slots[i:i + n]
        runs = []
        j = 0
        while j < n:
            k, t0 = chunk[j]
            j0 = j
            while j + 1 < n and chunk[j + 1] == (k, chunk[j][1] + 1):
                j += 1
            runs.append((k, t0, j - j0 + 1, j0))
            j += 1
        calls.append(dict(kind="g", t0=None, n=n, runs=runs))
        i += n
    return calls


def _preprocess(edge_index, x):
    """Host-side sharding: per-core, per-color padded gather-index tables."""
    dest = np.asarray(edge_index[0]).astype(np.int64)
    src = np.asarray(edge_index[1]).astype(np.int64)
    x = np.ascontiguousarray(np.asarray(x), dtype=np.float32)

    x_pack = np.zeros((RPACK, COLORS * D), np.float32)
    x_pack[:N_NODES // COLORS] = x.reshape(N_NODES // COLORS, COLORS * D)

    # Degree-balanced dest->core assignment: dests ranked by total in-degree
    # round-robin across cores, so every core sees the same degree profile
    # and the shared (max-over-cores) K schedule stays tight.
    total_deg = np.bincount(dest, minlength=N_NODES)
    rank = np.argsort(-total_deg, kind="stable")
    core_of_node = np.empty(N_NODES, np.int64)
    core_of_node[rank] = np.arange(N_NODES) % N_CORES
    dest_lists = [np.flatnonzero(core_of_node == c) for c in range(N_CORES)]
    local_id = np.empty(N_NODES, np.int64)
    for c in range(N_CORES):
        local_id[dest_lists[c]] = np.arange(len(dest_lists[c]))
    core_of = core_of_node[dest]
    # per (core, color): (perm, deg_pad, starts_pad, srcs_sorted)
    pc = [[None] * COLORS for _ in range(N_CORES)]
    K_all = np.zeros((N_CORES, COLORS, TILES), np.int64)
    for c in range(N_CORES):
        m = core_of == c
        d_loc = local_id[dest[m]]
        s_c = src[m]
        color = s_c % COLORS
        for q in range(COLORS):
            mq = color == q
            d_q = d_loc[mq]
            s_q = (s_c[mq] // COLORS).astype(np.int16)
            deg = np.bincount(d_q, minlength=NPC)
            order = np.argsort(d_q, kind="stable")
            s_sorted = s_q[order]
            starts = np.zeros(NPC, np.int64)
            starts[1:] = np.cumsum(deg)[:-1]
            perm = np.argsort(-deg, kind="stable")
            deg_pad = np.concatenate([deg[perm],
                                      np.zeros(NPC_PAD - NPC, np.int64)])
            starts_pad = np.concatenate([starts[perm],
                                         np.zeros(NPC_PAD - NPC, np.int64)])
            K_all[c, q] = deg_pad.reshape(TILES, P)[:, 0]
            pc[c][q] = (perm, deg_pad, starts_pad, s_sorted)

    K = K_all.max(axis=0)                      # [COLORS, TILES] shared schedule

    calls = [_make_calls(K[q]) for q in range(COLORS)]

    # Per-core idx tables: k-major rows per color (round 0 first), matching
    # the call schedule's slot order exactly (no padding slots).
    idx_maps = []
    n_slots_q = [int(K[q].sum()) for q in range(COLORS)]
    for c in range(N_CORES):
        parts = []
        for q in range(COLORS):
            perm, deg_pad, starts_pad, s_sorted = pc[c][q]
            kmax = int(K[q][0])
            if kmax == 0:
                continue
            s_safe = np.concatenate([s_sorted, np.zeros(1, np.int16)])
            kk = np.arange(kmax)[None, :]
            dg = deg_pad[:, None]
            st = starts_pad[:, None]
            pos = np.minimum(st + kk, len(s_safe) - 1)
            spread = ((np.arange(NPC_PAD)[:, None] + kk) % NZROWS) + DUMMY
            V = np.where(kk < dg, s_safe[pos], spread).astype(np.int16)
            Vt = V.reshape(TILES, P, kmax).transpose(2, 0, 1)  # [kmax,TILES,P]
            rows = np.empty((n_slots_q[q], P), np.int16)
            r = 0
            for k in range(kmax):
                n_k = int(np.count_nonzero(K[q] > k))
                rows[r:r + n_k] = Vt[k, :n_k]
                r += n_k
            parts.append(rows)
        vals = np.concatenate(parts, axis=0)   # [total_slots, P]
        # wrap: descriptor i of a call sits at partition i%16, col i//16,
        # replicated x8 across the 128 partitions.  Calls are concatenated
        # along the free dim so one contiguous DMA per color preloads them.
        cols = []
        r = 0
        for q in range(COLORS):
            for cl in calls[q]:
                n = cl["n"]
                blk = vals[r:r + n].reshape(n * P)
                wrapped = blk.reshape(n * P // 16, 16).T   # [16, n*8]
                cols.append(np.tile(wrapped, (8, 1)))      # [128, n*8]
                r += n
        idx_maps.append(np.ascontiguousarray(np.concatenate(cols, axis=1)))

    perms = [[pc[c][q][0] for q in range(COLORS)] for c in range(N_CORES)]
    K_key = tuple(tuple(int(v) for v in K[q]) for q in range(COLORS))
    return x_pack, idx_maps, perms, K_key, calls, dest_lists


def _build_program(K, calls):
    total_cols = sum(cl["n"] * 8 for q in range(COLORS) for cl in calls[q])
    nc = bacc.Bacc("TRN2", target_bir_lowering=False, debug=False,
                   num_devices=N_CORES, num_swdge_queues=4,
                   dynamic_dma_scratch_size=SCRATCH)
    x_dram = nc.dram_tensor("x", [RPACK, COLORS * D], mybir.dt.float32,
                            kind="ExternalInput")
    idx_dram = nc.dram_tensor("idx", [P, total_cols], mybir.dt.int16,
                              kind="ExternalInput")
    out_dram = nc.dram_tensor("out", [COLORS, NPC_PAD, D], mybir.dt.float32,
                              kind="ExternalOutput")

    # column offset of each call's idx block
    col_off = []
    off = 0
    for q in range(COLORS):
        offs = []
        for cl in calls[q]:
            offs.append(off)
            off += cl["n"] * 8
        col_off.append(offs)
    qcol = [col_off[q][0] for q in range(COLORS)] + [total_cols]

    with tile.TileContext(nc) as tc, ExitStack() as ctx:
        idx_pool = ctx.enter_context(tc.tile_pool(name="idx", bufs=1))
        g_pool = ctx.enter_context(tc.tile_pool(name="g", bufs=8))
        acc_pool = ctx.enter_context(tc.tile_pool(name="acc", bufs=2))

        # Preload idx tables per color so the first gather only waits on the
        # first chunk while the rest stream in behind it.
        idx_all = idx_pool.tile([P, total_cols], mybir.dt.int16,
                                tag="idx", name="idx_all")
        for q in range(COLORS):
            a, b = qcol[q], qcol[q + 1]
            if a == b:
                continue
            nc.sync.dma_start(out=idx_all[:, a:b], in_=idx_dram.ap()[:, a:b])

        # Warm up the SWDGE gather ucode while the idx tables stream in.
        warm_idx = idx_pool.tile([P, 8], mybir.dt.int16, tag="warm_idx",
                                 name="warm_idx")
        warm_g = g_pool.tile([P, 1, D], mybir.dt.float32, tag="warm_g",
                             name="warm_g", bufs=1)
        nc.gpsimd.memset(warm_idx[:], 0)
        prev = nc.gpsimd.dma_gather(
            out_ap=warm_g[:], in_ap=x_dram.ap()[:, 0:D],
            idxs_ap=warm_idx[:], num_idxs=P, num_idxs_reg=P,
            elem_size=D, elem_step=COLORS * D, queue_num=0,
            single_packet=SINGLE_PACKET)

        # Descriptor generation runs ASYNCHRONOUSLY on a per-queue SWDGE
        # worker (~8.6us per 1024-idx call); the Pool engine only blocks when
        # re-issuing to a still-busy queue, so rotating all 4 queues gives 4x
        # desc-gen throughput.  The 8 DMASW semaphore lanes are assigned in
        # FINAL schedule order and each lane's sem must only ever be updated
        # from one queue, so gathers are chained with no-sync deps (freezing
        # their order) and queue = (pool-DMA index) % 4, keeping lane L on
        # queue L % 4 forever.
        gi = 1                                 # warm gather was #0 (queue 0)
        for q in range(COLORS):
            acc = acc_pool.tile([P, TILES * D], mybir.dt.float32,
                                tag="acc", name=f"acc{q}")
            # Zero-degree tail tiles are complete from the start.
            for t in range(TILES):
                if K[q][t] == 0:
                    nc.vector.memset(acc[:, bass.ts(t, D)], 0.0)

            def store_chunk(a, b):
                nc.sync.dma_start(
                    out=out_dram.ap()[q].rearrange("(t p) d -> p t d", p=P)
                        [:, a:b],
                    in_=acc[:, a * D:b * D].rearrange("p (t d) -> p t d",
                                                      d=D))

            stored_from = TILES           # acc cols >= this are stored
            qcalls = calls[q]
            for ci, cl in enumerate(qcalls):
                n = cl["n"]
                nidx = n * P
                idxs_ap = idx_all[:, col_off[q][ci]:col_off[q][ci] + n * 8]
                if cl["kind"] == "acc0":
                    t0 = cl["t0"]
                    out_ap = acc[:, t0 * D:(t0 + n) * D].rearrange(
                        "p (s d) -> p s d", d=D)
                else:
                    g = g_pool.tile([P, n, D], mybir.dt.float32, tag="g",
                                    name=f"g{q}_{ci}")
                    out_ap = g[:]
                inst = nc.gpsimd.dma_gather(
                    out_ap=out_ap,
                    in_ap=x_dram.ap()[:, q * D:(q + 1) * D],
                    idxs_ap=idxs_ap,
                    num_idxs=nidx,
                    num_idxs_reg=nidx,
                    elem_size=D,
                    elem_step=COLORS * D,
                    queue_num=gi % 4,
                    single_packet=SINGLE_PACKET,
                )
                gi += 1
                deps = InstructionNameOrderedSet()
                deps.add(prev.ins.name)
                inst.ins.add_nosync_dependencies_from(deps)
                prev = inst
                if cl["kind"] == "g":
                    g2 = g[:].rearrange("p s d -> p (s d)")
                    for k, t0, L, j0 in cl["runs"]:
                        src_ap = g2[:, j0 * D:(j0 + L) * D]
                        dst_ap = acc[:, t0 * D:(t0 + L) * D]
                        if k == 0:
                            nc.vector.tensor_copy(dst_ap, src_ap)
                        else:
                            nc.vector.tensor_add(dst_ap, dst_ap, src_ap)
                # Stream out tile ranges as their last round completes
                # (k-major: high tiles finish first).
                if ci + 1 == len(qcalls):
                    if stored_from > 0:
                        store_chunk(0, stored_from)
                        stored_from = 0
                else:
                    nxt = qcalls[ci + 1]
                    if nxt["kind"] == "g" and nxt["runs"]:
                        k_next = nxt["runs"][0][0]
                        done_from = int(np.count_nonzero(
                            np.array(K[q]) > k_next))
                        if stored_from - done_from >= STORE_MIN_TILES:
                            store_chunk(done_from, stored_from)
                            stored_from = done_from
    nc.compile()
    return nc


def _install_profile_shim():
    """trace=True under axon needs the NTFF hook that this image's antenv
    lacks; register the ctypes-based one from trn_agent_boot."""
    import sys, types
    import concourse.bass_utils as bu
    if "antenv.axon_hooks" not in sys.modules:
        from trn_agent_boot.trn_boot import _ntff_profile_via_ctypes
        shim = types.ModuleType("antenv.axon_hooks")
        hook = _ntff_profile_via_ctypes("/opt/axon/libaxon_pjrt.so")
        shim.get_axon_ntff_profile_hook = lambda: hook
        shim.set_axon_ntff_profile_hook = lambda h: None
        sys.modules["antenv.axon_hooks"] = shim
    bu.upload_artifacts = lambda tmpdir: f"local:{tmpdir}"


def kernel(edge_index, x):
    global LAST_EXEC_NS, LAST_RESULTS
    (x_pack, idx_maps, perms, K, calls, dest_lists) = _preprocess(edge_index, x)

    cache_key = (K, S_MAX, DIRECT_ACC0, SINGLE_PACKET)
    if cache_key not in _COMPILE_CACHE:
        _COMPILE_CACHE[cache_key] = _build_program(K, calls)
    nc = _COMPILE_CACHE[cache_key]

    in_maps = [{"x": x_pack, "idx": idx_maps[c]} for c in range(N_CORES)]
    kwargs = {}
    if PROFILE:
        _install_profile_shim()
        kwargs = dict(trace=True, trace_cores=TRACE_CORES)
    res = run_bass_kernel_spmd(nc, in_maps, core_ids=list(range(N_CORES)),
                               **kwargs)
    LAST_EXEC_NS = res.exec_time_ns
    LAST_RESULTS = res

    out = np.empty((N_NODES, D), np.float32)
    for c in range(N_CORES):
        dev = res.results[c]["out"]            # [COLORS, NPC_PAD, D] bf16
        sl = np.zeros((NPC, D), np.float32)
        for q in range(COLORS):
            tmp = np.empty((NPC, D), np.float32)
            tmp[perms[c][q]] = dev[q][:NPC].astype(np.float32)
            sl += tmp
        out[dest_lists[c]] = sl
    return out


# revision 23
# speedup vs baseline: 3.6260x; 1.0082x over previous
"""GNN message passing (gather + segment-sum) on 8 Trainium2 NeuronCores.

Strategy (node-range sharding per the spec's sharding_hint):
  - Destination nodes are degree-balanced across the 8 cores (12500 nodes
    each), so each core owns a disjoint slice of the output and no
    cross-core reduction is needed.
  - The device-side gather uses the batched SWDGE row-gather
    (`nc.gpsimd.dma_gather`).  Its indices are int16, so x is packed as
    [25128, 256] (4 node rows per packed row plus zero rows) and
    source nodes are split into 4 "colors" by src % 4; color q gathers
    from column slice q*64:(q+1)*64 with elem_step=256 and index
    src//4 <= 25000.
  - Per core and color, the core's nodes are sorted by color-in-degree
    and grouped into 98 tiles of 128 (one SBUF partition per node).
    The slot schedule is K-MAJOR: round k touches the prefix of tiles
    whose max in-tile degree exceeds k, so consecutive slots are
    consecutive tiles and the accumulation is a handful of WIDE vector
    ops per gather call.
  - Round 0 gathers write DIRECTLY into the fp32 accumulator (no vector
    copy); rounds k>=1 gather into a staging tile and are added in.
  - Calls carry up to 32 slots (4096 descriptors = the full per-queue
    SWDGE ring at dynamic_dma_scratch_size=65536) to amortize the ~2us
    fixed SWDGE call overhead; call sizes are exact (no slot padding)
    and round-robin the 4 SWDGE queues so desc-gen of one call overlaps
    the SDMA drain of the previous ones.
  - Colors run sequentially and share a 2-deep accumulator pool; each
    color's finished tile ranges convert to bf16 and stream out to DRAM
    as their last round completes.  The host undoes the per-color
    degree-sort permutations, sums the color partials in fp32, and
    concatenates the 8 node-range slices.
"""

import numpy as np
from contextlib import ExitStack

import concourse.bacc as bacc
import concourse.bass as bass
import concourse.tile as tile
import concourse.mybir as mybir
from concourse.bass_utils import run_bass_kernel_spmd
from concourse.instruction_name_ordered_set import InstructionNameOrderedSet

N_NODES = 100000
N_EDGES = 1250000
D = 64
N_CORES = 8
NPC = N_NODES // N_CORES          # 12500 nodes per core
P = 128
TILES = (NPC + P - 1) // P        # 98 node tiles per core
NPC_PAD = TILES * P               # 12544
COLORS = 4
NZROWS = 128                      # zero rows dummies spread over (avoids
                                  # hotspotting one HBM line with pad reads)
RPACK = N_NODES // COLORS + NZROWS  # 25128 packed rows (tail = zeros)
DUMMY = N_NODES // COLORS         # first zero row
S_MAX = 8                         # max gather slots per dma_gather call:
                                  # single_packet coalesces a call into ONE
                                  # SDMA packet per engine, capped at 16KB =
                                  # 8 slots x 128 idx x 256B / 16 engines
SCRATCH = 16384                   # SWDGE ring carveout (default)
SINGLE_PACKET = True              # False (per-desc packets) measured ~6x
                                  # slower DMA; True is required for rate
STORE_MIN_TILES = 12              # min tile chunk for incremental writeback
DIRECT_ACC0 = True                # round-0 gathers write straight into acc

# Set by test.py for profiling; harness path leaves these untouched.
PROFILE = False
TRACE_CORES = None
LAST_EXEC_NS = None
LAST_RESULTS = None

_COMPILE_CACHE = {}


def _make_calls(K_q):
    """Call schedule for one color given its per-tile round counts K_q.

    Returns a list of calls; each call is a dict:
      kind: 'acc0' (round-0, gathers straight into the accumulator) or 'g'
      t0:   first tile (acc0 only)
      n:    slot count (descriptors = n*128)
      runs: [(k, t0, L, j0)] vector-add runs (g only)
    The slot order must match the idx-table row order (k-major).
    """
    K_q = np.asarray(K_q)
    kmax = int(K_q[0]) if len(K_q) else 0
    calls = []
    k_lo = 1 if DIRECT_ACC0 else 0
    if DIRECT_ACC0:
        # round 0: direct-to-acc calls over tiles [0, n0)
        n0 = int(np.count_nonzero(K_q > 0))
        t = 0
        while t < n0:
            n = min(S_MAX, n0 - t)
            calls.append(dict(kind="acc0", t0=t, n=n, runs=None))
            t += n
    # rounds k >= k_lo, k-major slot list
    slots = []
    for k in range(k_lo, kmax):
        n_k = int(np.count_nonzero(K_q > k))
        slots.extend((k, t) for t in range(n_k))
    i = 0
    while i < len(slots):
        n = min(S_MAX, len(slots) - i)
        chunk = slots[i:i + n]
        runs = []
        j = 0
        while j < n:
            k, t0 = chunk[j]
            j0 = j
            while j + 1 < n and chunk[j + 1] == (k, chunk[j][1] + 1):
                j += 1
            runs.append((k, t0, j - j0 + 1, j0))
            j += 1
        calls.append(dict(kind="g", t0=None, n=n, runs=runs))
        i += n
    return calls


def _preprocess(edge_index, x):
    """Host-side sharding: per-core, per-color padded gather-index tables."""
    dest = np.asarray(edge_index[0]).astype(np.int64)
    src = np.asarray(edge_index[1]).astype(np.int64)
    x = np.ascontiguousarray(np.asarray(x), dtype=np.float32)

    x_pack = np.zeros((RPACK, COLORS * D), np.float32)
    x_pack[:N_NODES // COLORS] = x.reshape(N_NODES // COLORS, COLORS * D)

    # Degree-balanced dest->core assignment: dests ranked by total in-degree
    # round-robin across cores, so every core sees the same degree profile
    # and the shared (max-over-cores) K schedule stays tight.
    total_deg = np.bincount(dest, minlength=N_NODES)
    rank = np.argsort(-total_deg, kind="stable")
    core_of_node = np.empty(N_NODES, np.int64)
    core_of_node[rank] = np.arange(N_NODES) % N_CORES
    dest_lists = [np.flatnonzero(core_of_node == c) for c in range(N_CORES)]
    local_id = np.empty(N_NODES, np.int64)
    for c in range(N_CORES):
        local_id[dest_lists[c]] = np.arange(len(dest_lists[c]))
    core_of = core_of_node[dest]
    # per (core, color): (perm, deg_pad, starts_pad, srcs_sorted)
    pc = [[None] * COLORS for _ in range(N_CORES)]
    K_all = np.zeros((N_CORES, COLORS, TILES), np.int64)
    for c in range(N_CORES):
        m = core_of == c
        d_loc = local_id[dest[m]]
        s_c = src[m]
        color = s_c % COLORS
        for q in range(COLORS):
            mq = color == q
            d_q = d_loc[mq]
            s_q = (s_c[mq] // COLORS).astype(np.int16)
            deg = np.bincount(d_q, minlength=NPC)
            order = np.argsort(d_q, kind="stable")
            s_sorted = s_q[order]
            starts = np.zeros(NPC, np.int64)
            starts[1:] = np.cumsum(deg)[:-1]
            perm = np.argsort(-deg, kind="stable")
            deg_pad = np.concatenate([deg[perm],
                                      np.zeros(NPC_PAD - NPC, np.int64)])
            starts_pad = np.concatenate([starts[perm],
                                         np.zeros(NPC_PAD - NPC, np.int64)])
            K_all[c, q] = deg_pad.reshape(TILES, P)[:, 0]
            pc[c][q] = (perm, deg_pad, starts_pad, s_sorted)

    K = K_all.max(axis=0)                      # [COLORS, TILES] shared schedule

    calls = [_make_calls(K[q]) for q in range(COLORS)]

    # Per-core idx tables: k-major rows per color (round 0 first), matching
    # the call schedule's slot order exactly (no padding slots).
    idx_maps = []
    n_slots_q = [int(K[q].sum()) for q in range(COLORS)]
    for c in range(N_CORES):
        parts = []
        for q in range(COLORS):
            perm, deg_pad, starts_pad, s_sorted = pc[c][q]
            kmax = int(K[q][0])
            if kmax == 0:
                continue
            s_safe = np.concatenate([s_sorted, np.zeros(1, np.int16)])
            kk = np.arange(kmax)[None, :]
            dg = deg_pad[:, None]
            st = starts_pad[:, None]
            pos = np.minimum(st + kk, len(s_safe) - 1)
            spread = ((np.arange(NPC_PAD)[:, None] + kk) % NZROWS) + DUMMY
            V = np.where(kk < dg, s_safe[pos], spread).astype(np.int16)
            Vt = V.reshape(TILES, P, kmax).transpose(2, 0, 1)  # [kmax,TILES,P]
            rows = np.empty((n_slots_q[q], P), np.int16)
            r = 0
            for k in range(kmax):
                n_k = int(np.count_nonzero(K[q] > k))
                rows[r:r + n_k] = Vt[k, :n_k]
                r += n_k
            parts.append(rows)
        vals = np.concatenate(parts, axis=0)   # [total_slots, P]
        # wrap: descriptor i of a call sits at partition i%16, col i//16,
        # replicated x8 across the 128 partitions.  Calls are concatenated
        # along the free dim so one contiguous DMA per color preloads them.
        cols = []
        r = 0
        for q in range(COLORS):
            for cl in calls[q]:
                n = cl["n"]
                blk = vals[r:r + n].reshape(n * P)
                wrapped = blk.reshape(n * P // 16, 16).T   # [16, n*8]
                cols.append(np.tile(wrapped, (8, 1)))      # [128, n*8]
                r += n
        idx_maps.append(np.ascontiguousarray(np.concatenate(cols, axis=1)))

    perms = [[pc[c][q][0] for q in range(COLORS)] for c in range(N_CORES)]
    K_key = tuple(tuple(int(v) for v in K[q]) for q in range(COLORS))
    return x_pack, idx_maps, perms, K_key, calls, dest_lists


def _build_program(K, calls):
    total_cols = sum(cl["n"] * 8 for q in range(COLORS) for cl in calls[q])
    nc = bacc.Bacc("TRN2", target_bir_lowering=False, debug=False,
                   num_devices=N_CORES, num_swdge_queues=4,
                   dynamic_dma_scratch_size=SCRATCH)
    x_dram = nc.dram_tensor("x", [RPACK, COLORS * D], mybir.dt.float32,
                            kind="ExternalInput")
    idx_dram = nc.dram_tensor("idx", [P, total_cols], mybir.dt.int16,
                              kind="ExternalInput")
    out_dram = nc.dram_tensor("out", [COLORS, NPC_PAD, D], mybir.dt.float32,
                              kind="ExternalOutput")

    # column offset of each call's idx block
    col_off = []
    off = 0
    for q in range(COLORS):
        offs = []
        for cl in calls[q]:
            offs.append(off)
            off += cl["n"] * 8
        col_off.append(offs)
    qcol = [col_off[q][0] for q in range(COLORS)] + [total_cols]

    with tile.TileContext(nc) as tc, ExitStack() as ctx:
        idx_pool = ctx.enter_context(tc.tile_pool(name="idx", bufs=1))
        g_pool = ctx.enter_context(tc.tile_pool(name="g", bufs=8))
        acc_pool = ctx.enter_context(tc.tile_pool(name="acc", bufs=2))

        # Preload idx tables per color so the first gather only waits on the
        # first chunk while the rest stream in behind it.
        idx_all = idx_pool.tile([P, total_cols], mybir.dt.int16,
                                tag="idx", name="idx_all")
        # Color 0's first two calls preload separately so the first gather
        # only waits on a 16KB slice, not the whole color-0 table.
        head = min(2, len(calls[0]))
        cut = col_off[0][head - 1] + calls[0][head - 1]["n"] * 8 \
            if head else 0
        pre = [(0, cut), (cut, qcol[1])] + \
              [(qcol[q], qcol[q + 1]) for q in range(1, COLORS)]
        for a, b in pre:
            if a == b:
                continue
            nc.sync.dma_start(out=idx_all[:, a:b], in_=idx_dram.ap()[:, a:b])

        # Warm up the SWDGE gather ucode while the idx tables stream in.
        warm_idx = idx_pool.tile([P, 8], mybir.dt.int16, tag="warm_idx",
                                 name="warm_idx")
        warm_g = g_pool.tile([P, 1, D], mybir.dt.float32, tag="warm_g",
                             name="warm_g", bufs=1)
        nc.gpsimd.memset(warm_idx[:], 0)
        prev = nc.gpsimd.dma_gather(
            out_ap=warm_g[:], in_ap=x_dram.ap()[:, 0:D],
            idxs_ap=warm_idx[:], num_idxs=P, num_idxs_reg=P,
            elem_size=D, elem_step=COLORS * D, queue_num=0,
            single_packet=SINGLE_PACKET)

        # Descriptor generation runs ASYNCHRONOUSLY on a per-queue SWDGE
        # worker (~8.6us per 1024-idx call); the Pool engine only blocks when
        # re-issuing to a still-busy queue, so rotating all 4 queues gives 4x
        # desc-gen throughput.  The 8 DMASW semaphore lanes are assigned in
        # FINAL schedule order and each lane's sem must only ever be updated
        # from one queue, so gathers are chained with no-sync deps (freezing
        # their order) and queue = (pool-DMA index) % 4, keeping lane L on
        # queue L % 4 forever.
        gi = 1                                 # warm gather was #0 (queue 0)
        for q in range(COLORS):
            acc = acc_pool.tile([P, TILES * D], mybir.dt.float32,
                                tag="acc", name=f"acc{q}")
            # Zero-degree tail tiles are complete from the start.
            for t in range(TILES):
                if K[q][t] == 0:
                    nc.vector.memset(acc[:, bass.ts(t, D)], 0.0)

            def store_chunk(a, b):
                nc.sync.dma_start(
                    out=out_dram.ap()[q].rearrange("(t p) d -> p t d", p=P)
                        [:, a:b],
                    in_=acc[:, a * D:b * D].rearrange("p (t d) -> p t d",
                                                      d=D))

            stored_from = TILES           # acc cols >= this are stored
            qcalls = calls[q]
            for ci, cl in enumerate(qcalls):
                n = cl["n"]
                nidx = n * P
                idxs_ap = idx_all[:, col_off[q][ci]:col_off[q][ci] + n * 8]
                if cl["kind"] == "acc0":
                    t0 = cl["t0"]
                    out_ap = acc[:, t0 * D:(t0 + n) * D].rearrange(
                        "p (s d) -> p s d", d=D)
                else:
                    g = g_pool.tile([P, n, D], mybir.dt.float32, tag="g",
                                    name=f"g{q}_{ci}")
                    out_ap = g[:]
                inst = nc.gpsimd.dma_gather(
                    out_ap=out_ap,
                    in_ap=x_dram.ap()[:, q * D:(q + 1) * D],
                    idxs_ap=idxs_ap,
                    num_idxs=nidx,
                    num_idxs_reg=nidx,
                    elem_size=D,
                    elem_step=COLORS * D,
                    queue_num=gi % 4,
                    single_packet=SINGLE_PACKET,
                )
                gi += 1
                deps = InstructionNameOrderedSet()
                deps.add(prev.ins.name)
                inst.ins.add_nosync_dependencies_from(deps)
                prev = inst
                if cl["kind"] == "g":
                    g2 = g[:].rearrange("p s d -> p (s d)")
                    for k, t0, L, j0 in cl["runs"]:
                        src_ap = g2[:, j0 * D:(j0 + L) * D]
                        dst_ap = acc[:, t0 * D:(t0 + L) * D]
                        if k == 0:
                            nc.vector.tensor_copy(dst_ap, src_ap)
                        else:
                            nc.vector.tensor_add(dst_ap, dst_ap, src_ap)
                # Stream out tile ranges as their last round completes
                # (k-major: high tiles finish first).
                if ci + 1 == len(qcalls):
                    if stored_from > 0:
                        store_chunk(0, stored_from)
                        stored_from = 0
                else:
                    nxt = qcalls[ci + 1]
                    if nxt["kind"] == "g" and nxt["runs"]:
                        k_next = nxt["runs"][0][0]
                        done_from = int(np.count_nonzero(
                            np.array(K[q]) > k_next))
                        if stored_from - done_from >= STORE_MIN_TILES:
                            store_chunk(done_from, stored_from)
                            stored_from = done_from
    nc.compile()
    return nc


def _install_profile_shim():
    """trace=True under axon needs the NTFF hook that this image's antenv
    lacks; register the ctypes-based one from trn_agent_boot."""
    import sys, types
    import concourse.bass_utils as bu
    if "antenv.axon_hooks" not in sys.modules:
        from trn_agent_boot.trn_boot import _ntff_profile_via_ctypes
        shim = types.ModuleType("antenv.axon_hooks")
        hook = _ntff_profile_via_ctypes("/opt/axon/libaxon_pjrt.so")
        shim.get_axon_ntff_profile_hook = lambda: hook
        shim.set_axon_ntff_profile_hook = lambda h: None
        sys.modules["antenv.axon_hooks"] = shim
    bu.upload_artifacts = lambda tmpdir: f"local:{tmpdir}"


def kernel(edge_index, x):
    global LAST_EXEC_NS, LAST_RESULTS
    (x_pack, idx_maps, perms, K, calls, dest_lists) = _preprocess(edge_index, x)

    cache_key = (K, S_MAX, DIRECT_ACC0, SINGLE_PACKET)
    if cache_key not in _COMPILE_CACHE:
        _COMPILE_CACHE[cache_key] = _build_program(K, calls)
    nc = _COMPILE_CACHE[cache_key]

    in_maps = [{"x": x_pack, "idx": idx_maps[c]} for c in range(N_CORES)]
    kwargs = {}
    if PROFILE:
        _install_profile_shim()
        kwargs = dict(trace=True, trace_cores=TRACE_CORES)
    res = run_bass_kernel_spmd(nc, in_maps, core_ids=list(range(N_CORES)),
                               **kwargs)
    LAST_EXEC_NS = res.exec_time_ns
    LAST_RESULTS = res

    out = np.empty((N_NODES, D), np.float32)
    for c in range(N_CORES):
        dev = res.results[c]["out"]            # [COLORS, NPC_PAD, D] bf16
        sl = np.zeros((NPC, D), np.float32)
        for q in range(COLORS):
            tmp = np.empty((NPC, D), np.float32)
            tmp[perms[c][q]] = dev[q][:NPC].astype(np.float32)
            sl += tmp
        out[dest_lists[c]] = sl
    return out
